# revision 7
# baseline (speedup 1.0000x reference)
"""Trainium2 Bass kernel for the gated core/periphery Conv2d problem.

Computation (matches the reference):
  core_out = conv2d(x, core3x3, pad=1)                        # [B,256,56,56]
  xs       = sum_c x                                          # [B,1,56,56]
  periph   = conv2d(xs, 5x5 stencil w/ zero 3x3 center, pad=2)
  gate     = sigmoid(scale * (core_out - thresh[c]))
  out      = core_out + gate * periph

Strategy: data-parallel over batch, 4 images per NeuronCore (8 cores).
Per core, per image:
  - x is zero-padded by 2 on host, cast to bf16 -> SBUF [128ch, 60, 60].
  - core conv: for each (half of 256 out-ch) x (7 row-tiles of 8 rows):
    9 accumulating matmuls (K=128 in-ch, M=128 out-ch, N=8*56=448) over
    shifted views of the padded input. fp32 PSUM accumulation.
  - xs: ones-vector matmul over the padded image (8 chunks of 450).
  - periphery: shifted copies of xs on 25 partitions (one per 5x5 tap),
    single matmul per row-tile with lhsT = tap-weights broadcast to all
    128 out-channel lanes (free partition-broadcast of the periph map).
  - epilogue: ACT sigmoid (scale/bias fused), 2 DVE ops, DMA out fp32.
"""

import os

import numpy as np

# matmul input precision: "bf16" (inputs rounded to bf16), "f32r" (fp32
# storage, reduced-precision PE multiply, same 1 cycle/row at N>=256),
# "f32" (exact, 4 cycles/row). PSUM accumulation is fp32 in every mode.
MM_DT = os.environ.get("KERNEL_MM_DT", "bf16")

B, CIN, COUT, H, W = 32, 128, 256, 56, 56
NCORES = 8
BPC = B // NCORES            # images per core
PAD = 2
HP, WP = H + 2 * PAD, W + 2 * PAD      # 60, 60
RT = 8                        # output rows per tile
NTILES = H // RT              # 7
NF = RT * W                   # 448 free elems per matmul
XS_CH = 450                   # xs chunk (8 * 450 = 3600 = HP*WP)
XSH_LEN = (H - 1) * WP + W    # 3356: flattened span needed by row tiles


def _build_program(scale_val: float):
    import concourse.bass as bass
    import concourse.mybir as mybir
    import concourse.tile as tile

    bf16 = mybir.dt.bfloat16
    f32 = mybir.dt.float32

    nc = bass.Bass("TRN2", target_bir_lowering=False, debug=False)

    xp = nc.dram_tensor("xp", [BPC, CIN, HP, WP], bf16, kind="ExternalInput").ap()
    wT = nc.dram_tensor("wT", [9, CIN, COUT], bf16, kind="ExternalInput").ap()
    pkb = nc.dram_tensor("pkb", [25, 128], bf16, kind="ExternalInput").ap()
    ones = nc.dram_tensor("ones", [CIN, 1], bf16, kind="ExternalInput").ap()
    biasg = nc.dram_tensor("biasg", [128, 2], f32, kind="ExternalInput").ap()
    y = nc.dram_tensor("y", [BPC, COUT, H, W], f32, kind="ExternalOutput").ap()

    sig = mybir.ActivationFunctionType.Sigmoid

    with tile.TileContext(nc) as tc:
        with (
            tc.tile_pool(name="consts", bufs=1) as consts,
            tc.tile_pool(name="xpool", bufs=2) as xpool,
            tc.tile_pool(name="spool", bufs=2) as spool,
            tc.tile_pool(name="epool", bufs=3) as epool,
            tc.tile_pool(name="psA", bufs=4, space="PSUM") as psA_pool,
            tc.tile_pool(name="psS", bufs=2, space="PSUM") as psS_pool,
        ):
            # input-side DMAs ride the Activation-engine HWDGE queue so they
            # never sit behind the output writes in the SP queue's FIFO
            ones_sb = consts.tile([CIN, 1], bf16)
            nc.scalar.dma_start(out=ones_sb, in_=ones)
            wT_sb = consts.tile([CIN, 9, COUT], bf16)
            nc.scalar.dma_start(out=wT_sb, in_=wT.rearrange("t k m -> k t m"))
            pkb_sb = consts.tile([25, 128], bf16)
            nc.scalar.dma_start(out=pkb_sb, in_=pkb)
            bias_sb = consts.tile([128, 2], f32)
            nc.scalar.dma_start(out=bias_sb, in_=biasg)

            # prefetch every image up front; PE never waits at image bounds
            x_pads = []
            for b in range(BPC):
                xpb = xpool.tile([CIN, HP, WP], bf16, tag=f"x_pad{b}", bufs=1)
                nc.scalar.dma_start(out=xpb, in_=xp[b])
                x_pads.append(xpb)

            for b in range(BPC):
                x_pad = x_pads[b]
                xflat = x_pad.rearrange("k a b -> k (a b)")

                # channel sum of the padded image -> xs_sb [1, 3600] bf16
                xs_sb = spool.tile([1, HP * WP], bf16, tag="xs_sb")
                for c in range(HP * WP // XS_CH):
                    psx = psS_pool.tile([1, XS_CH], f32, tag="ps_small")
                    nc.tensor.matmul(
                        psx,
                        lhsT=ones_sb,
                        rhs=xflat[:, XS_CH * c : XS_CH * (c + 1)],
                        start=True,
                        stop=True,
                    )
                    nc.scalar.copy(out=xs_sb[:, XS_CH * c : XS_CH * (c + 1)], in_=psx)

                # 25 shifted copies of xs (one per 5x5 tap) on 25 partitions
                xsh = spool.tile([25, XSH_LEN], bf16, tag="xsh")
                for r in range(5):
                    src = bass.AP(
                        tensor=xs_sb.tensor,
                        offset=xs_sb.offset + r * WP,
                        ap=[list(xs_sb.ap[0]), [1, 5], [1, XSH_LEN]],
                    )
                    nc.scalar.dma_start(out=xsh[5 * r : 5 * r + 5, :], in_=src)

                for ti in range(NTILES):
                    i0 = ti * RT
                    # periphery map for this row tile, broadcast to 128 lanes
                    psP = psS_pool.tile([128, RT, W], f32, tag="ps_small")
                    rhsP = bass.AP(
                        tensor=xsh.tensor,
                        offset=xsh.offset + i0 * WP,
                        ap=[list(xsh.ap[0]), [WP, RT], [1, W]],
                    )
                    nc.tensor.matmul(psP, lhsT=pkb_sb, rhs=rhsP, start=True, stop=True)

                    for h in range(2):
                        psA = psA_pool.tile([128, RT, W], f32, tag="psA")
                        for t9 in range(9):
                            kh, kw = divmod(t9, 3)
                            nc.tensor.matmul(
                                psA,
                                lhsT=wT_sb[:, t9, 128 * h : 128 * (h + 1)],
                                rhs=x_pad[
                                    :, i0 + kh + 1 : i0 + kh + 1 + RT, kw + 1 : kw + 1 + W
                                ],
                                start=(t9 == 0),
                                stop=(t9 == 8),
                            )
                        gate = epool.tile([128, RT, W], f32, tag="gate")
                        nc.scalar.activation(
                            out=gate,
                            in_=psA,
                            func=sig,
                            bias=bias_sb[:, h : h + 1],
                            scale=scale_val,
                        )
                        t1 = epool.tile([128, RT, W], f32, tag="t1")
                        nc.vector.tensor_mul(t1, gate, psP)
                        out_sb = epool.tile([128, RT, W], f32, tag="out_sb")
                        nc.vector.tensor_add(out_sb, t1, psA)
                        nc.sync.dma_start(
                            out=y[b, 128 * h : 128 * (h + 1), i0 : i0 + RT, :],
                            in_=out_sb,
                        )
    return nc


def _split_multiwait_drains(nc, limit: int = 1) -> int:
    """This walrus build rejects instructions carrying more than one semaphore
    wait ("Too many sync wait commands" in setupSyncWait). Move excess waits
    onto single-wait NoOps issued just before on the same engine — in-order
    per engine, so the synchronization semantics are preserved."""
    import concourse.mybir as mybir

    n_split = 0
    for f in nc.m.functions:
        for blk in f.blocks:
            out = []
            changed = False
            for inst in blk.instructions:
                si = inst.sync_info
                if si is not None and si.on_wait and len(si.on_wait) > limit:
                    waits = list(si.on_wait)
                    for w in waits[:-limit]:
                        n_split += 1
                        d = mybir.InstNoOp(
                            name=f"{inst.name}-swait{n_split}", ins=[], outs=[]
                        )
                        d.engine = inst.engine
                        d.sync_info = mybir.SyncInfo(on_wait=[w], on_update=[])
                        out.append(d)
                    inst.sync_info = mybir.SyncInfo(
                        on_wait=waits[-limit:], on_update=list(si.on_update or [])
                    )
                    changed = True
                out.append(inst)
            if changed:
                blk.instructions = out
    return n_split


def _host_inputs(x, core, periphery, thresh, scale):
    import ml_dtypes

    bf16 = ml_dtypes.bfloat16
    scale_val = float(np.asarray(scale).reshape(-1)[0])

    xp_full = np.pad(
        np.asarray(x, np.float32), ((0, 0), (0, 0), (PAD, PAD), (PAD, PAD))
    ).astype(bf16)

    # per-tap transposed core weights: wT[t, cin, cout]
    wT = (
        np.asarray(core, np.float32)
        .transpose(2, 3, 1, 0)
        .reshape(9, CIN, COUT)
        .astype(bf16)
    )

    # 5x5 periphery stencil with zero 3x3 center, broadcast over 128 lanes
    periph_mask = np.ones((5, 5), dtype=bool)
    periph_mask[1:4, 1:4] = False
    p_full = np.zeros(25, np.float32)
    p_full[np.flatnonzero(periph_mask.reshape(-1))] = np.asarray(
        periphery, np.float32
    )
    pkb = np.ascontiguousarray(
        np.broadcast_to(p_full[:, None], (25, 128))
    ).astype(bf16)

    ones = np.ones((CIN, 1), bf16)
    biasg = np.ascontiguousarray(
        (-scale_val * np.asarray(thresh, np.float32)).reshape(2, 128).T
    ).astype(np.float32)

    common = {"wT": wT, "pkb": pkb, "ones": ones, "biasg": biasg}
    in_maps = [
        {"xp": np.ascontiguousarray(xp_full[c * BPC : (c + 1) * BPC]), **common}
        for c in range(NCORES)
    ]
    return in_maps, scale_val


def kernel(x, core, periphery, thresh, scale):
    from concourse import bass_utils

    in_maps, scale_val = _host_inputs(x, core, periphery, thresh, scale)
    nc = _build_program(scale_val)
    _split_multiwait_drains(nc)
    res = bass_utils.run_bass_kernel_spmd(nc, in_maps, core_ids=list(range(NCORES)))
    out = np.concatenate(
        [res.results[c]["y"] for c in range(NCORES)], axis=0
    ).astype(np.float32)
    return out


# revision 13
# speedup vs baseline: 1.2442x; 1.2442x over previous
"""Trainium2 Bass kernel for the gated core/periphery Conv2d problem.

Computation (matches the reference):
  core_out = conv2d(x, core3x3, pad=1)                        # [B,256,56,56]
  xs       = sum_c x                                          # [B,1,56,56]
  periph   = conv2d(xs, 5x5 stencil w/ zero 3x3 center, pad=2)
  gate     = sigmoid(scale * (core_out - thresh[c]))
  out      = core_out + gate * periph

Strategy: data-parallel over batch, 4 images per NeuronCore (8 cores).
Per core, per image:
  - x is zero-padded by 2 on host, cast to bf16 -> SBUF [128ch, 60, 60].
  - core conv: for each (half of 256 out-ch) x (7 row-tiles of 8 rows):
    9 accumulating matmuls (K=128 in-ch, M=128 out-ch, N=8*56=448) over
    shifted views of the padded input. fp32 PSUM accumulation.
  - xs: ones-vector matmul over the padded image (8 chunks of 450).
  - periphery: shifted copies of xs on 25 partitions (one per 5x5 tap),
    single matmul per row-tile with lhsT = tap-weights broadcast to all
    128 out-channel lanes (free partition-broadcast of the periph map).
  - epilogue: ACT sigmoid (scale/bias fused), 2 DVE ops, DMA out fp32.
"""

import os

import numpy as np

# matmul input precision: "bf16" (inputs rounded to bf16), "f32r" (fp32
# storage, reduced-precision PE multiply, same 1 cycle/row at N>=256),
# "f32" (exact, 4 cycles/row). PSUM accumulation is fp32 in every mode.
MM_DT = os.environ.get("KERNEL_MM_DT", "bf16")

B, CIN, COUT, H, W = 32, 128, 256, 56, 56
NCORES = 8
BPC = B // NCORES            # images per core
PAD = 2
HP, WP = H + 2 * PAD, W + 2 * PAD      # 60, 60
RT = 8                        # output rows per tile
NTILES = H // RT              # 7
NF = RT * W                   # 448 free elems per matmul
XS_CH = 450                   # xs chunk (8 * 450 = 3600 = HP*WP)
XSH_LEN = (H - 1) * WP + W    # 3356: flattened span needed by row tiles


def _build_program(scale_val: float, mm_dt: str = MM_DT):
    import concourse.bass as bass
    import concourse.mybir as mybir
    import concourse.tile as tile

    f32 = mybir.dt.float32
    bf16 = mybir.dt.bfloat16 if mm_dt == "bf16" else f32  # matmul storage dtype
    if mm_dt == "f32r":
        mmc = lambda ap: ap.bitcast(mybir.dt.float32r)
    else:
        mmc = lambda ap: ap

    nc = bass.Bass("TRN2", target_bir_lowering=False, debug=False)

    xp = nc.dram_tensor("xp", [BPC, CIN, HP, WP], bf16, kind="ExternalInput").ap()
    wT = nc.dram_tensor("wT", [9, CIN, COUT], bf16, kind="ExternalInput").ap()
    pkb = nc.dram_tensor("pkb", [25, 128], bf16, kind="ExternalInput").ap()
    ones = nc.dram_tensor("ones", [CIN, 1], bf16, kind="ExternalInput").ap()
    biasg = nc.dram_tensor("biasg", [128, 2], f32, kind="ExternalInput").ap()
    y = nc.dram_tensor("y", [BPC, COUT, H, W], f32, kind="ExternalOutput").ap()

    sig = mybir.ActivationFunctionType.Sigmoid

    with tile.TileContext(nc) as tc:
        with (
            tc.tile_pool(name="consts", bufs=1) as consts,
            tc.tile_pool(name="xpool", bufs=2) as xpool,
            tc.tile_pool(name="spool", bufs=2) as spool,
            tc.tile_pool(name="epool", bufs=3) as epool,
            tc.tile_pool(name="psA", bufs=4, space="PSUM") as psA_pool,
            tc.tile_pool(name="psP", bufs=2, space="PSUM") as psP_pool,
            tc.tile_pool(name="psX", bufs=2, space="PSUM") as psX_pool,
        ):
            # input-side DMAs ride the Activation-engine HWDGE queue so they
            # never sit behind the output writes in the SP queue's FIFO.
            # Order: what the PE needs first loads first.
            ones_sb = consts.tile([CIN, 1], bf16)
            nc.scalar.dma_start(out=ones_sb, in_=ones)
            x_pads = []
            for b in range(BPC):
                x_pads.append(
                    xpool.tile(
                        [CIN, HP, WP],
                        bf16,
                        tag=f"x_pad{b}",
                        bufs=1,
                        name=f"x_pad{b}",
                    )
                )
            nc.scalar.dma_start(out=x_pads[0], in_=xp[0])
            wT_sb = consts.tile([CIN, 9, COUT], bf16)
            nc.scalar.dma_start(out=wT_sb, in_=wT.rearrange("t k m -> k t m"))
            pkb_sb = consts.tile([25, 128], bf16)
            nc.scalar.dma_start(out=pkb_sb, in_=pkb)
            bias_sb = consts.tile([128, 2], f32)
            nc.scalar.dma_start(out=bias_sb, in_=biasg)
            for b in range(1, BPC):
                nc.scalar.dma_start(out=x_pads[b], in_=xp[b])

            def emit_xs_phase(b):
                """Channel-sum image b and build the 25 shifted tap copies.
                PE cost is tiny; interleaved inside the previous image's core
                conv so the chain never stalls the PE at an image boundary."""
                xflat = x_pads[b].rearrange("k a b -> k (a b)")
                xs_sb = spool.tile([1, HP * WP], bf16, tag="xs_sb")
                for c in range(HP * WP // XS_CH):
                    psx = psX_pool.tile([1, XS_CH], f32, tag="psX")
                    nc.tensor.matmul(
                        psx,
                        lhsT=mmc(ones_sb),
                        rhs=mmc(xflat[:, XS_CH * c : XS_CH * (c + 1)]),
                        start=True,
                        stop=True,
                    )
                    nc.vector.tensor_copy(
                        xs_sb[:, XS_CH * c : XS_CH * (c + 1)], psx
                    )
                xsh = spool.tile([25, XSH_LEN], bf16, tag="xsh")
                for r in range(5):
                    src = bass.AP(
                        tensor=xs_sb.tensor,
                        offset=xs_sb.offset + r * WP,
                        ap=[list(xs_sb.ap[0]), [1, 5], [1, XSH_LEN]],
                    )
                    nc.scalar.dma_start(out=xsh[5 * r : 5 * r + 5, :], in_=src)
                return xsh

            xshs = {0: emit_xs_phase(0)}
            for b in range(BPC):
                x_pad = x_pads[b]
                xsh = xshs[b]
                for ti in range(NTILES):
                    if ti == 2 and b + 1 < BPC:
                        xshs[b + 1] = emit_xs_phase(b + 1)
                    i0 = ti * RT
                    # periphery map for this row tile, broadcast to 128 lanes
                    psP = psP_pool.tile([128, RT, W], f32, tag="psP")
                    rhsP = bass.AP(
                        tensor=xsh.tensor,
                        offset=xsh.offset + i0 * WP,
                        ap=[list(xsh.ap[0]), [WP, RT], [1, W]],
                    )
                    nc.tensor.matmul(
                        psP, lhsT=mmc(pkb_sb), rhs=mmc(rhsP), start=True, stop=True
                    )

                    for h in range(2):
                        psA = psA_pool.tile([128, RT, W], f32, tag="psA")
                        for t9 in range(9):
                            kh, kw = divmod(t9, 3)
                            nc.tensor.matmul(
                                psA,
                                lhsT=mmc(wT_sb[:, t9, 128 * h : 128 * (h + 1)]),
                                rhs=mmc(
                                    x_pad[
                                        :,
                                        i0 + kh + 1 : i0 + kh + 1 + RT,
                                        kw + 1 : kw + 1 + W,
                                    ]
                                ),
                                start=(t9 == 0),
                                stop=(t9 == 8),
                            )
                        gate = epool.tile([128, RT, W], f32, tag="gate")
                        nc.scalar.activation(
                            out=gate,
                            in_=psA,
                            func=sig,
                            bias=bias_sb[:, h : h + 1],
                            scale=scale_val,
                        )
                        t1 = epool.tile([128, RT, W], f32, tag="t1")
                        nc.vector.tensor_mul(t1, gate, psP)
                        out_sb = epool.tile([128, RT, W], f32, tag="out_sb")
                        nc.vector.tensor_add(out_sb, t1, psA)
                        nc.sync.dma_start(
                            out=y[b, 128 * h : 128 * (h + 1), i0 : i0 + RT, :],
                            in_=out_sb,
                        )
    return nc


def _split_multiwait_drains(nc, limit: int = 1) -> int:
    """This walrus build rejects instructions carrying more than one semaphore
    wait ("Too many sync wait commands" in setupSyncWait). Move excess waits
    onto single-wait NoOps issued just before on the same engine — in-order
    per engine, so the synchronization semantics are preserved."""
    import concourse.mybir as mybir

    n_split = 0
    for f in nc.m.functions:
        for blk in f.blocks:
            out = []
            changed = False
            for inst in blk.instructions:
                si = inst.sync_info
                if si is not None and si.on_wait and len(si.on_wait) > limit:
                    waits = list(si.on_wait)
                    for w in waits[:-limit]:
                        n_split += 1
                        d = mybir.InstNoOp(
                            name=f"{inst.name}-swait{n_split}", ins=[], outs=[]
                        )
                        d.engine = inst.engine
                        d.sync_info = mybir.SyncInfo(on_wait=[w], on_update=[])
                        out.append(d)
                    inst.sync_info = mybir.SyncInfo(
                        on_wait=waits[-limit:], on_update=list(si.on_update or [])
                    )
                    changed = True
                out.append(inst)
            if changed:
                blk.instructions = out
    return n_split


def _host_inputs(x, core, periphery, thresh, scale):
    import ml_dtypes

    bf16 = ml_dtypes.bfloat16
    scale_val = float(np.asarray(scale).reshape(-1)[0])

    xp_full = np.pad(
        np.asarray(x, np.float32), ((0, 0), (0, 0), (PAD, PAD), (PAD, PAD))
    ).astype(bf16)

    # per-tap transposed core weights: wT[t, cin, cout]
    wT = (
        np.asarray(core, np.float32)
        .transpose(2, 3, 1, 0)
        .reshape(9, CIN, COUT)
        .astype(bf16)
    )

    # 5x5 periphery stencil with zero 3x3 center, broadcast over 128 lanes
    periph_mask = np.ones((5, 5), dtype=bool)
    periph_mask[1:4, 1:4] = False
    p_full = np.zeros(25, np.float32)
    p_full[np.flatnonzero(periph_mask.reshape(-1))] = np.asarray(
        periphery, np.float32
    )
    pkb = np.ascontiguousarray(
        np.broadcast_to(p_full[:, None], (25, 128))
    ).astype(bf16)

    ones = np.ones((CIN, 1), bf16)
    biasg = np.ascontiguousarray(
        (-scale_val * np.asarray(thresh, np.float32)).reshape(2, 128).T
    ).astype(np.float32)

    common = {"wT": wT, "pkb": pkb, "ones": ones, "biasg": biasg}
    in_maps = [
        {"xp": np.ascontiguousarray(xp_full[c * BPC : (c + 1) * BPC]), **common}
        for c in range(NCORES)
    ]
    return in_maps, scale_val


def kernel(x, core, periphery, thresh, scale):
    from concourse import bass_utils

    in_maps, scale_val = _host_inputs(x, core, periphery, thresh, scale)
    nc = _build_program(scale_val)
    _split_multiwait_drains(nc)
    res = bass_utils.run_bass_kernel_spmd(nc, in_maps, core_ids=list(range(NCORES)))
    out = np.concatenate(
        [res.results[c]["y"] for c in range(NCORES)], axis=0
    ).astype(np.float32)
    return out
